# revision 10
# baseline (speedup 1.0000x reference)
"""Trainium2 Bass kernel for a pre-norm decoder block (B=4, S=1024, D=1024,
H=16, DK=DV=64, DM=4096), data-parallel over 8 NeuronCores.

Sharding: core c handles batch b = c//2 and query rows [q0, q0+512) with
q0 = (c%2)*512.  Every core recomputes LayerNorm+K/V over the full sequence
of its batch element (zero-communication causal attention); the causal mask
arrives as per-core 0/1 input data so the program is uniform SPMD.

Layout strategy: activations are kept feature-major ("X.T", contraction dim
on partitions) for all matmuls; attention is computed fully transposed
(S.T = K Q^T with keys on partitions) so softmax sums become matmuls against
an extra ones-column appended to V.  LayerNorm stats run token-major via
bn_stats, and 128x128 PE transposes convert between the two layouts.
"""

import os
import sys

for _p in ("/opt/trn_rl_repo", "/root/.axon_site/_ro/trn_rl_repo"):
    if os.path.isdir(_p) and _p not in sys.path:
        sys.path.insert(0, _p)

from contextlib import ExitStack

import numpy as np

import concourse.bass as bass
import concourse.mybir as mybir
import concourse.tile as tile
from concourse.masks import make_identity
from concourse.vector_clock import ScopedClock, VectorClock

B, S, D = 4, 1024, 1024
H, DK, DV = 16, 64, 64
DM = 4096
EPS = 1e-5
P = 128
SQ = 512                      # queries per core
NCORES = 8
NT = S // P                   # 8 token tiles over the full sequence
NQT = SQ // P                 # 4 token tiles over this core's queries
ND = D // P                   # 8 feature tiles of D
NM = DM // P                  # 32 feature tiles of DM
F32 = mybir.dt.float32
F32R = mybir.dt.float32r
AF = mybir.ActivationFunctionType
ALU = mybir.AluOpType


class _SplitDrainTC(tile.TileContext):
    """The walrus build in this container rejects instructions carrying many
    sem waits ("Too many sync wait commands" on Tile's tail Drain).  Split the
    tail-drain waits across several drain instructions, a few procs each."""

    _CHUNK = 4

    def _drain_and_barrier(self, tick_clock, wait_clock):
        gc = tick_clock.global_clock
        n = len(gc)
        for i in range(0, n, self._CHUNK):
            part = VectorClock(
                [gc[p] if i <= p < i + self._CHUNK else 0 for p in range(n)]
            )
            di = self.nc.sync.drain()
            wait_clock.add_sem_waits(di.ins, ScopedClock({None: part}))
        self.nc.all_engine_barrier()
        assert self.sems is not None
        popped = self.nc._tile_sem_poison_stack.pop()
        assert popped is self._sem_poison
        self.nc.clear_and_free_semaphores(list(self.sems.allocated().values()))
        self.nc.all_engine_barrier()


def _r(ap):
    return ap.bitcast(F32R)


def _split_sync_waits(nc, limit=1):
    """walrus in this container rejects instructions with more than `limit`
    sem waits ("Too many sync wait commands").  Hoist surplus waits onto
    ENGINE_NOP carriers inserted just before the instruction on the same
    engine stream (engine execution is in-order, so this is equivalent)."""
    from bass_rust import SyncInfo

    nop_op = nc.isa.Opcode.NEURON_ISA_TPB_OPCODE_NOP
    for fn in nc.m.functions:
        for bb in fn.blocks:
            insts = bb.instructions
            out = []
            changed = False
            for inst in insts:
                si = inst.sync_info
                waits = list(si.on_wait) if si and si.on_wait else []
                if len(waits) > limit:
                    extra, keep = waits[:-limit], waits[-limit:]
                    for j in range(0, len(extra), limit):
                        nop = nc.engines[inst.engine]._isa(nop_op, {})
                        nop.sync_info = SyncInfo(on_wait=extra[j:j + limit],
                                                 on_update=[])
                        out.append(nop)
                    si.on_wait = keep
                    changed = True
                out.append(inst)
            if changed:
                insts.clear()
                insts.extend(out)


def build_program():
    nc = bass.Bass(target_bir_lowering=False)

    xkv = nc.declare_dram_parameter("xkv", [S, D], F32, isOutput=False)
    xq = nc.declare_dram_parameter("xq", [SQ, D], F32, isOutput=False)
    mask = nc.declare_dram_parameter("mask", [NT, P, SQ], F32, isOutput=False)
    wq = nc.declare_dram_parameter("wq", [D, H * DK], F32, isOutput=False)
    wk = nc.declare_dram_parameter("wk", [D, H * DK], F32, isOutput=False)
    wv = nc.declare_dram_parameter("wv", [D, H * DV], F32, isOutput=False)
    wo = nc.declare_dram_parameter("wo", [H * DV, D], F32, isOutput=False)
    w1 = nc.declare_dram_parameter("w1", [D, DM], F32, isOutput=False)
    w2 = nc.declare_dram_parameter("w2", [DM, D], F32, isOutput=False)
    # per-partition column layouts: value for feature f sits at [f % 128, f // 128]
    g1c = nc.declare_dram_parameter("g1c", [P, ND], F32, isOutput=False)
    be1c = nc.declare_dram_parameter("be1c", [P, ND], F32, isOutput=False)
    g2c = nc.declare_dram_parameter("g2c", [P, ND], F32, isOutput=False)
    be2c = nc.declare_dram_parameter("be2c", [P, ND], F32, isOutput=False)
    b1c = nc.declare_dram_parameter("b1c", [P, NM], F32, isOutput=False)
    b2c = nc.declare_dram_parameter("b2c", [P, ND], F32, isOutput=False)
    y = nc.declare_dram_parameter("y", [SQ, D], F32, isOutput=True)

    # DRAM views
    xkv_t = xkv[:].rearrange("(tt p) d -> tt p d", p=P)        # [8,128,1024]
    xq_t = xq[:].rearrange("(tt p) d -> tt p d", p=P)          # [4,128,1024]
    y_t = y[:].rearrange("(tt p) d -> tt p d", p=P)
    wq_v = wq[:].rearrange("(dt p) f -> p dt f", p=P)          # [128,8,1024]
    wk_v = wk[:].rearrange("(dt p) f -> p dt f", p=P)
    wv_v = wv[:].rearrange("(dt p) f -> p dt f", p=P)
    wo_v = wo[:].rearrange("(ct p) f -> p ct f", p=P)
    w1_v = w1[:].rearrange("(dt p) f -> p dt f", p=P)          # [128,8,4096]
    w2_v = w2[:].rearrange("(mt p) f -> p mt f", p=P)          # [128,32,1024]

    with _SplitDrainTC(nc) as tc, ExitStack() as ctx:
        const = ctx.enter_context(tc.tile_pool(name="const", bufs=1))
        io4 = ctx.enter_context(tc.tile_pool(name="io4", bufs=4))
        pt3 = ctx.enter_context(tc.tile_pool(name="pt3", bufs=3))
        bc2 = ctx.enter_context(tc.tile_pool(name="bc2", bufs=2))
        wstream = ctx.enter_context(tc.tile_pool(name="wstream", bufs=2))
        small = ctx.enter_context(tc.tile_pool(name="small", bufs=4))
        psum_mm = ctx.enter_context(tc.tile_pool(name="psum_mm", bufs=3, space="PSUM"))
        psum_ctx = ctx.enter_context(tc.tile_pool(name="psum_ctx", bufs=2, space="PSUM"))
        psum_tr = ctx.enter_context(tc.tile_pool(name="psum_tr", bufs=3, space="PSUM"))

        ident = const.tile([P, P], F32)
        make_identity(nc, ident)
        eps_t = const.tile([P, 1], F32)
        nc.vector.memset(eps_t, EPS)
        ones1 = const.tile([1, DV], F32)
        nc.vector.memset(ones1, 1.0)
        vones = const.tile([P, NT, H], F32)
        nc.vector.memset(vones, 1.0)
        g1s = const.tile([P, ND], F32)
        be1s = const.tile([P, ND], F32)
        g2s = const.tile([P, ND], F32)
        be2s = const.tile([P, ND], F32)
        b1s = const.tile([P, NM], F32)
        b2s = const.tile([P, ND], F32)
        for dst, src in ((g1s, g1c), (be1s, be1c), (g2s, g2c), (be2s, be2c),
                         (b1s, b1c), (b2s, b2c)):
            nc.sync.dma_start(out=dst[:], in_=src[:])

        def layernorm_tile(x_sb, nrm, rows):
            """token-major LN: x_sb [128, D*] -> nrm [128, D*] (no gain/bias)."""
            d = x_sb.shape[-1]
            nsub = d // 512
            stats = small.tile([P, nsub, 6], F32, tag="stats")
            xv = x_sb.rearrange("p (n f) -> p n f", f=512)
            for sg in range(nsub):
                nc.vector.bn_stats(out=stats[:rows, sg, :], in_=xv[:rows, sg, :])
            mv = small.tile([P, 2], F32, tag="mv")
            nc.vector.bn_aggr(out=mv[:rows], in_=stats[:rows])
            # mv[:,1] <- 1/sqrt(var+eps)
            nc.scalar.activation(out=mv[:rows, 1:2], in_=mv[:rows, 1:2],
                                 func=AF.Sqrt, bias=eps_t[:rows], scale=1.0)
            nc.vector.reciprocal(out=mv[:rows, 1:2], in_=mv[:rows, 1:2])
            nc.vector.tensor_scalar(
                out=nrm[:rows], in0=x_sb[:rows],
                scalar1=mv[:rows, 0:1], scalar2=mv[:rows, 1:2],
                op0=ALU.subtract, op1=ALU.mult)

        def transpose_block(dst, src_block, gcol, bcol):
            """PE-transpose one [128,128] SBUF block into dst (SBUF, feature-
            major) applying per-feature gain/bias columns on the way out."""
            ps = psum_tr.tile([P, P], F32, tag="tr")
            nc.tensor.transpose(ps[:], src_block, ident[:])
            nc.vector.tensor_scalar(out=dst, in0=ps[:], scalar1=gcol,
                                    scalar2=bcol, op0=ALU.mult, op1=ALU.add)

        ctxT = ctx.enter_context(tc.tile_pool(name="p_ctxT", bufs=1))
        ctxT_t = ctxT.tile([P, ND, SQ], F32R)               # ctx.T [(h dv), q]

        with ExitStack() as qkv_ctx:
            pqkv = qkv_ctx.enter_context(tc.tile_pool(name="p_qkv", bufs=1))
            KT = pqkv.tile([P, ND, S], F32R)                # K.T [(h dk), t]
            QT = pqkv.tile([P, ND, SQ], F32R)               # Q.T/8 [(h dk), q]
            Vaug = pqkv.tile([P, NT, H, DV + 1], F32R)      # V tokens-major + ones col

            with ExitStack() as ln_ctx:
                pln = ln_ctx.enter_context(tc.tile_pool(name="p_ln", bufs=1))
                nT = pln.tile([P, ND, S], F32R)             # LN1(xkv).T
                nqT = pln.tile([P, ND, SQ], F32R)           # LN1(xq).T

                # ---- stage A: LayerNorm1 + transpose to feature-major ----
                for tt in range(NT):
                    x_sb = io4.tile([P, D], F32, tag="xt")
                    nc.sync.dma_start(out=x_sb[:], in_=xkv_t[tt])
                    nrm = io4.tile([P, D], F32, tag="xt")
                    layernorm_tile(x_sb, nrm, P)
                    for dt in range(ND):
                        transpose_block(nT[:, dt, tt * P:(tt + 1) * P],
                                        nrm[:, dt * P:(dt + 1) * P],
                                        g1s[:, dt:dt + 1], be1s[:, dt:dt + 1])
                for tt in range(NQT):
                    x_sb = io4.tile([P, D], F32, tag="xt")
                    nc.sync.dma_start(out=x_sb[:], in_=xq_t[tt])
                    nrm = io4.tile([P, D], F32, tag="xt")
                    layernorm_tile(x_sb, nrm, P)
                    for dt in range(ND):
                        transpose_block(nqT[:, dt, tt * P:(tt + 1) * P],
                                        nrm[:, dt * P:(dt + 1) * P],
                                        g1s[:, dt:dt + 1], be1s[:, dt:dt + 1])

                # ---- stage B: Q/K/V projections ----
                for ft in range(ND):
                    wq_sb = wstream.tile([P, ND, P], F32R, tag="w128")
                    nc.sync.dma_start(out=wq_sb[:], in_=wq_v[:, :, ft * P:(ft + 1) * P].bitcast(F32R))
                    ps = psum_mm.tile([P, SQ], F32, tag="mm")
                    for dt in range(ND):
                        nc.tensor.matmul(ps[:], _r(wq_sb[:, dt, :]), _r(nqT[:, dt, :]),
                                         start=dt == 0, stop=dt == ND - 1)
                    # fold the 1/sqrt(DK) score scale into Q.T
                    nc.scalar.mul(out=QT[:, ft, :], in_=ps[:], mul=1.0 / np.sqrt(DK))

                    wk_sb = wstream.tile([P, ND, P], F32R, tag="w128")
                    nc.sync.dma_start(out=wk_sb[:], in_=wk_v[:, :, ft * P:(ft + 1) * P].bitcast(F32R))
                    for th in range(S // SQ):
                        ps = psum_mm.tile([P, SQ], F32, tag="mm")
                        for dt in range(ND):
                            nc.tensor.matmul(ps[:], _r(wk_sb[:, dt, :]),
                                             _r(nT[:, dt, th * SQ:(th + 1) * SQ]),
                                             start=dt == 0, stop=dt == ND - 1)
                        nc.scalar.copy(out=KT[:, ft, th * SQ:(th + 1) * SQ], in_=ps[:])

                # V token-major: out[t, (h dv)] in quarters of 256 features
                for fq in range(4):
                    wv_sb = wstream.tile([P, ND, 256], F32R, tag="w8k")
                    nc.sync.dma_start(out=wv_sb[:], in_=wv_v[:, :, fq * 256:(fq + 1) * 256].bitcast(F32R))
                    for tt in range(NT):
                        ps = psum_mm.tile([P, SQ], F32, tag="mm")
                        for dt in range(ND):
                            nc.tensor.matmul(ps[:, :256], _r(nT[:, dt, tt * P:(tt + 1) * P]),
                                             _r(wv_sb[:, dt, :]),
                                             start=dt == 0, stop=dt == ND - 1)
                        nc.scalar.copy(
                            out=Vaug[:, tt, fq * 4:(fq + 1) * 4, 0:DV],
                            in_=ps[:, :256].rearrange("p (h e) -> p h e", e=DV))

            nc.vector.tensor_copy(out=Vaug[:, :, :, DV:DV + 1],
                                  in_=vones[:, :, :, None])

            # ---- stage C: attention, fully transposed ----
            with ExitStack() as attn_ctx:
                pmask = attn_ctx.enter_context(tc.tile_pool(name="p_mask", bufs=1))
                mask_sb = pmask.tile([P, NT, SQ], F32)
                nc.sync.dma_start(out=mask_sb[:],
                                  in_=mask[:].rearrange("kt p q -> p kt q"))
                for h in range(H):
                    hp = (h % 2) * 64
                    ff = h // 2
                    ps_c = psum_ctx.tile([DV + 1, SQ], F32, tag="ctx")
                    for kt in range(NT):
                        ps_s = psum_mm.tile([P, SQ], F32, tag="mm")
                        nc.tensor.matmul(
                            ps_s[:],
                            _r(KT[hp:hp + 64, ff, kt * P:(kt + 1) * P]),
                            _r(QT[hp:hp + 64, ff, :]),
                            start=True, stop=True)
                        pT = pt3.tile([P, SQ], F32R, tag="pt")
                        nc.scalar.activation(out=pT[:], in_=ps_s[:], func=AF.Exp)
                        nc.gpsimd.tensor_mul(pT[:], pT[:], mask_sb[:, kt, :])
                        nc.tensor.matmul(ps_c[:], _r(Vaug[:, kt, h, :]), _r(pT[:]),
                                         start=kt == 0, stop=kt == NT - 1)
                    rz = small.tile([1, SQ], F32, tag="rz")
                    nc.vector.reciprocal(out=rz[:], in_=ps_c[DV:DV + 1, :])
                    # broadcast rz across 64 partitions: ones[1,64].T @ rz[1,SQ]
                    ps_bc = psum_tr.tile([DV, SQ], F32, tag="tr")
                    nc.tensor.matmul(ps_bc[:], ones1[:], rz[:], start=True, stop=True)
                    rzb = bc2.tile([DV, SQ], F32, tag="rzb")
                    nc.scalar.copy(out=rzb[:], in_=ps_bc[:])
                    nc.vector.tensor_mul(ctxT_t[hp:hp + 64, ff, :],
                                         ps_c[0:DV, :], rzb[:])

        # ---- stages D/E/F: Wo + residual, LN2, FFN ----
        with ExitStack() as tail_ctx:
            ptail = tail_ctx.enter_context(tc.tile_pool(name="p_tail", bufs=1))
            aoT = ptail.tile([P, ND, SQ], F32)
            x1 = ptail.tile([P, NQT, D], F32)              # token-major x + attn
            n2T = ptail.tile([P, ND, SQ], F32R)
            h1T = ptail.tile([P, NM // 2, SQ], F32R)        # half of relu(h).T
            fT = ptail.tile([P, ND, SQ], F32)              # ffn output .T

            # D: attn_out.T = Wo.T @ ctx.T
            for ft in range(ND):
                wo_sb = wstream.tile([P, ND, P], F32R, tag="w128")
                nc.sync.dma_start(out=wo_sb[:], in_=wo_v[:, :, ft * P:(ft + 1) * P].bitcast(F32R))
                ps = psum_mm.tile([P, SQ], F32, tag="mm")
                for ct in range(ND):
                    nc.tensor.matmul(ps[:], _r(wo_sb[:, ct, :]), _r(ctxT_t[:, ct, :]),
                                     start=ct == 0, stop=ct == ND - 1)
                nc.scalar.copy(out=aoT[:, ft, :], in_=ps[:])
            # transpose back + residual -> x1 (token-major)
            for tt in range(NQT):
                xq_sb = io4.tile([P, D], F32, tag="xt")
                nc.sync.dma_start(out=xq_sb[:], in_=xq_t[tt])
                for ft in range(ND):
                    ps = psum_tr.tile([P, P], F32, tag="tr")
                    nc.tensor.transpose(ps[:], aoT[:, ft, tt * P:(tt + 1) * P], ident[:])
                    nc.vector.tensor_add(out=x1[:, tt, ft * P:(ft + 1) * P],
                                         in0=ps[:], in1=xq_sb[:, ft * P:(ft + 1) * P])

            # E: LayerNorm2 -> n2T (feature-major)
            for tt in range(NQT):
                nrm = io4.tile([P, D], F32, tag="xt")
                layernorm_tile(x1[:, tt, :], nrm, P)
                for dt in range(ND):
                    transpose_block(n2T[:, dt, tt * P:(tt + 1) * P],
                                    nrm[:, dt * P:(dt + 1) * P],
                                    g2s[:, dt:dt + 1], be2s[:, dt:dt + 1])

            # F: FFN in two DM halves to bound SBUF
            for half in range(2):
                mt0 = half * (NM // 2)
                for mi in range(NM // 2):
                    mt = mt0 + mi
                    w1_sb = wstream.tile([P, ND, P], F32R, tag="w128")
                    nc.sync.dma_start(out=w1_sb[:], in_=w1_v[:, :, mt * P:(mt + 1) * P].bitcast(F32R))
                    ps = psum_mm.tile([P, SQ], F32, tag="mm")
                    for dt in range(ND):
                        nc.tensor.matmul(ps[:], _r(w1_sb[:, dt, :]), _r(n2T[:, dt, :]),
                                         start=dt == 0, stop=dt == ND - 1)
                    nc.scalar.activation(out=h1T[:, mi, :], in_=ps[:], func=AF.Relu,
                                         bias=b1s[:, mt:mt + 1])
                for ft in range(ND):
                    w2_sb = wstream.tile([P, NM // 2, P], F32R, tag="w8k")
                    nc.sync.dma_start(
                        out=w2_sb[:],
                        in_=w2_v[:, mt0:mt0 + NM // 2, ft * P:(ft + 1) * P].bitcast(F32R))
                    ps = psum_mm.tile([P, SQ], F32, tag="mm")
                    for mi in range(NM // 2):
                        nc.tensor.matmul(ps[:], _r(w2_sb[:, mi, :]), _r(h1T[:, mi, :]),
                                         start=mi == 0, stop=mi == NM // 2 - 1)
                    if half == 0:
                        nc.vector.tensor_scalar(out=fT[:, ft, :], in0=ps[:],
                                                scalar1=b2s[:, ft:ft + 1], scalar2=None,
                                                op0=ALU.add)
                    else:
                        nc.vector.tensor_add(out=fT[:, ft, :], in0=fT[:, ft, :],
                                             in1=ps[:])

            # transpose back + residual -> y
            for tt in range(NQT):
                y_sb = io4.tile([P, D], F32, tag="xt")
                for ft in range(ND):
                    ps = psum_tr.tile([P, P], F32, tag="tr")
                    nc.tensor.transpose(ps[:], fT[:, ft, tt * P:(tt + 1) * P], ident[:])
                    nc.vector.tensor_add(out=y_sb[:, ft * P:(ft + 1) * P],
                                         in0=ps[:], in1=x1[:, tt, ft * P:(ft + 1) * P])
                nc.sync.dma_start(out=y_t[tt], in_=y_sb[:])

    return nc


_NC_CACHE = None


def _get_program():
    # The wait-split pass is HW-only: CoreSim's event loop cannot execute the
    # inserted carrier nops, so sim users call build_program() directly.
    global _NC_CACHE
    if _NC_CACHE is None:
        nc = build_program()
        _split_sync_waits(nc)
        _NC_CACHE = nc
    return _NC_CACHE


def make_in_maps(inputs):
    x = np.ascontiguousarray(np.asarray(inputs["x"], dtype=np.float32))
    Wq = np.asarray(inputs["Wq"], dtype=np.float32)
    Wk = np.asarray(inputs["Wk"], dtype=np.float32)
    Wv = np.asarray(inputs["Wv"], dtype=np.float32)
    wq_f = np.ascontiguousarray(Wq.transpose(1, 0, 2).reshape(D, H * DK))
    wk_f = np.ascontiguousarray(Wk.transpose(1, 0, 2).reshape(D, H * DK))
    wv_f = np.ascontiguousarray(Wv.transpose(1, 0, 2).reshape(D, H * DV))
    wo = np.ascontiguousarray(np.asarray(inputs["Wo"], dtype=np.float32))
    w1 = np.ascontiguousarray(np.asarray(inputs["w1"], dtype=np.float32))
    w2 = np.ascontiguousarray(np.asarray(inputs["w2"], dtype=np.float32))

    def col(v, n):
        return np.ascontiguousarray(
            np.asarray(v, dtype=np.float32).reshape(n, P).T)

    g1c, be1c = col(inputs["g1"], ND), col(inputs["be1"], ND)
    g2c, be2c = col(inputs["g2"], ND), col(inputs["be2"], ND)
    b1c, b2c = col(inputs["b1"], NM), col(inputs["b2"], ND)

    kk = np.arange(S).reshape(NT, P)
    masks = {}
    for q0 in (0, SQ):
        qq = q0 + np.arange(SQ)
        masks[q0] = np.ascontiguousarray(
            (qq[None, None, :] >= kk[:, :, None]).astype(np.float32))

    in_maps = []
    for c in range(NCORES):
        b, q0 = c // 2, (c % 2) * SQ
        in_maps.append({
            "xkv": x[b], "xq": np.ascontiguousarray(x[b, q0:q0 + SQ]),
            "mask": masks[q0],
            "wq": wq_f, "wk": wk_f, "wv": wv_f, "wo": wo,
            "w1": w1, "w2": w2,
            "g1c": g1c, "be1c": be1c, "g2c": g2c, "be2c": be2c,
            "b1c": b1c, "b2c": b2c,
        })
    return in_maps


def kernel(**inputs):
    from concourse.bass_utils import run_bass_kernel_spmd

    nc = _get_program()
    in_maps = make_in_maps(inputs)
    res = run_bass_kernel_spmd(nc, in_maps, list(range(NCORES)))
    y = np.empty((B, S, D), dtype=np.float32)
    for c in range(NCORES):
        b, q0 = c // 2, (c % 2) * SQ
        y[b, q0:q0 + SQ] = res.results[c]["y"]
    return y


# revision 11
# speedup vs baseline: 1.2773x; 1.2773x over previous
"""Trainium2 Bass kernel for a pre-norm decoder block (B=4, S=1024, D=1024,
H=16, DK=DV=64, DM=4096), data-parallel over 8 NeuronCores.

Sharding: core c handles batch b = c//2 and query rows [q0, q0+512) with
q0 = (c%2)*512.  Every core recomputes LayerNorm+K/V over the full sequence
of its batch element (zero-communication causal attention); the causal mask
arrives as per-core 0/1 input data so the program is uniform SPMD.

Layout strategy: activations are kept feature-major ("X.T", contraction dim
on partitions) for all matmuls; attention is computed fully transposed
(S.T = K Q^T with keys on partitions) so softmax sums become matmuls against
an extra ones-column appended to V.  LayerNorm stats run token-major via
bn_stats, and 128x128 PE transposes convert between the two layouts.
"""

import os
import sys

for _p in ("/opt/trn_rl_repo", "/root/.axon_site/_ro/trn_rl_repo"):
    if os.path.isdir(_p) and _p not in sys.path:
        sys.path.insert(0, _p)

from contextlib import ExitStack

import numpy as np

import concourse.bass as bass
import concourse.mybir as mybir
import concourse.tile as tile
from concourse.masks import make_identity
from concourse.vector_clock import ScopedClock, VectorClock

B, S, D = 4, 1024, 1024
H, DK, DV = 16, 64, 64
DM = 4096
EPS = 1e-5
P = 128
SQ = 512                      # queries per core
NCORES = 8
NT = S // P                   # 8 token tiles over the full sequence
NQT = SQ // P                 # 4 token tiles over this core's queries
ND = D // P                   # 8 feature tiles of D
NM = DM // P                  # 32 feature tiles of DM
F32 = mybir.dt.float32
F32R = mybir.dt.float32r
AF = mybir.ActivationFunctionType
ALU = mybir.AluOpType


class _SplitDrainTC(tile.TileContext):
    """The walrus build in this container rejects instructions carrying many
    sem waits ("Too many sync wait commands" on Tile's tail Drain).  Split the
    tail-drain waits across several drain instructions, a few procs each."""

    _CHUNK = 4

    def _drain_and_barrier(self, tick_clock, wait_clock):
        gc = tick_clock.global_clock
        n = len(gc)
        for i in range(0, n, self._CHUNK):
            part = VectorClock(
                [gc[p] if i <= p < i + self._CHUNK else 0 for p in range(n)]
            )
            di = self.nc.sync.drain()
            wait_clock.add_sem_waits(di.ins, ScopedClock({None: part}))
        self.nc.all_engine_barrier()
        assert self.sems is not None
        popped = self.nc._tile_sem_poison_stack.pop()
        assert popped is self._sem_poison
        self.nc.clear_and_free_semaphores(list(self.sems.allocated().values()))
        self.nc.all_engine_barrier()


def _r(ap):
    return ap.bitcast(F32R)


def _split_sync_waits(nc, limit=1):
    """walrus in this container rejects instructions with more than `limit`
    sem waits ("Too many sync wait commands").  Hoist surplus waits onto
    ENGINE_NOP carriers inserted just before the instruction on the same
    engine stream (engine execution is in-order, so this is equivalent)."""
    from bass_rust import SyncInfo

    nop_op = nc.isa.Opcode.NEURON_ISA_TPB_OPCODE_NOP
    for fn in nc.m.functions:
        for bb in fn.blocks:
            insts = bb.instructions
            out = []
            changed = False
            for inst in insts:
                si = inst.sync_info
                waits = list(si.on_wait) if si and si.on_wait else []
                if len(waits) > limit:
                    extra, keep = waits[:-limit], waits[-limit:]
                    for j in range(0, len(extra), limit):
                        nop = nc.engines[inst.engine]._isa(nop_op, {})
                        nop.sync_info = SyncInfo(on_wait=extra[j:j + limit],
                                                 on_update=[])
                        out.append(nop)
                    si.on_wait = keep
                    changed = True
                out.append(inst)
            if changed:
                insts.clear()
                insts.extend(out)


def build_program():
    nc = bass.Bass(target_bir_lowering=False)

    xkv = nc.declare_dram_parameter("xkv", [S, D], F32, isOutput=False)
    xq = nc.declare_dram_parameter("xq", [SQ, D], F32, isOutput=False)
    mask = nc.declare_dram_parameter("mask", [P, NT, SQ], F32, isOutput=False)
    # weights arrive pre-packed so every SBUF weight tile is one fully
    # contiguous HBM range (512-byte-strided loads are DMA-descriptor bound)
    wq = nc.declare_dram_parameter("wq", [ND, P, ND, P], F32, isOutput=False)
    wk = nc.declare_dram_parameter("wk", [ND, P, ND, P], F32, isOutput=False)
    wv = nc.declare_dram_parameter("wv", [4, P, ND, 256], F32, isOutput=False)
    wo = nc.declare_dram_parameter("wo", [ND, P, ND, P], F32, isOutput=False)
    w1 = nc.declare_dram_parameter("w1", [NM, P, ND, P], F32, isOutput=False)
    w2 = nc.declare_dram_parameter("w2", [ND, P, NM, P], F32, isOutput=False)
    # per-partition column layouts: value for feature f sits at [f % 128, f // 128]
    g1c = nc.declare_dram_parameter("g1c", [P, ND], F32, isOutput=False)
    be1c = nc.declare_dram_parameter("be1c", [P, ND], F32, isOutput=False)
    g2c = nc.declare_dram_parameter("g2c", [P, ND], F32, isOutput=False)
    be2c = nc.declare_dram_parameter("be2c", [P, ND], F32, isOutput=False)
    b1c = nc.declare_dram_parameter("b1c", [P, NM], F32, isOutput=False)
    b2c = nc.declare_dram_parameter("b2c", [P, ND], F32, isOutput=False)
    y = nc.declare_dram_parameter("y", [SQ, D], F32, isOutput=True)

    # DRAM views
    xkv_t = xkv[:].rearrange("(tt p) d -> tt p d", p=P)        # [8,128,1024]
    xq_t = xq[:].rearrange("(tt p) d -> tt p d", p=P)          # [4,128,1024]
    y_t = y[:].rearrange("(tt p) d -> tt p d", p=P)


    with _SplitDrainTC(nc) as tc, ExitStack() as ctx:
        const = ctx.enter_context(tc.tile_pool(name="const", bufs=1))
        io4 = ctx.enter_context(tc.tile_pool(name="io4", bufs=4))
        pt3 = ctx.enter_context(tc.tile_pool(name="pt3", bufs=3))
        bc2 = ctx.enter_context(tc.tile_pool(name="bc2", bufs=2))
        wstream = ctx.enter_context(tc.tile_pool(name="wstream", bufs=2))
        small = ctx.enter_context(tc.tile_pool(name="small", bufs=4))
        psum_mm = ctx.enter_context(tc.tile_pool(name="psum_mm", bufs=3, space="PSUM"))
        psum_ctx = ctx.enter_context(tc.tile_pool(name="psum_ctx", bufs=2, space="PSUM"))
        psum_tr = ctx.enter_context(tc.tile_pool(name="psum_tr", bufs=3, space="PSUM"))

        ident = const.tile([P, P], F32)
        make_identity(nc, ident)
        eps_t = const.tile([P, 1], F32)
        nc.vector.memset(eps_t, EPS)
        ones1 = const.tile([1, DV], F32)
        nc.vector.memset(ones1, 1.0)
        vones = const.tile([P, NT, H], F32)
        nc.vector.memset(vones, 1.0)
        g1s = const.tile([P, ND], F32)
        be1s = const.tile([P, ND], F32)
        g2s = const.tile([P, ND], F32)
        be2s = const.tile([P, ND], F32)
        b1s = const.tile([P, NM], F32)
        b2s = const.tile([P, ND], F32)
        for dst, src in ((g1s, g1c), (be1s, be1c), (g2s, g2c), (be2s, be2c),
                         (b1s, b1c), (b2s, b2c)):
            nc.sync.dma_start(out=dst[:], in_=src[:])

        def layernorm_tile(x_sb, nrm, rows):
            """token-major LN: x_sb [128, D*] -> nrm [128, D*] (no gain/bias)."""
            d = x_sb.shape[-1]
            nsub = d // 512
            stats = small.tile([P, nsub, 6], F32, tag="stats")
            xv = x_sb.rearrange("p (n f) -> p n f", f=512)
            for sg in range(nsub):
                nc.vector.bn_stats(out=stats[:rows, sg, :], in_=xv[:rows, sg, :])
            mv = small.tile([P, 2], F32, tag="mv")
            nc.vector.bn_aggr(out=mv[:rows], in_=stats[:rows])
            # mv[:,1] <- 1/sqrt(var+eps)
            nc.scalar.activation(out=mv[:rows, 1:2], in_=mv[:rows, 1:2],
                                 func=AF.Sqrt, bias=eps_t[:rows], scale=1.0)
            nc.vector.reciprocal(out=mv[:rows, 1:2], in_=mv[:rows, 1:2])
            nc.vector.tensor_scalar(
                out=nrm[:rows], in0=x_sb[:rows],
                scalar1=mv[:rows, 0:1], scalar2=mv[:rows, 1:2],
                op0=ALU.subtract, op1=ALU.mult)

        def transpose_block(dst, src_block, gcol, bcol):
            """PE-transpose one [128,128] SBUF block into dst (SBUF, feature-
            major) applying per-feature gain/bias columns on the way out."""
            ps = psum_tr.tile([P, P], F32, tag="tr")
            nc.tensor.transpose(ps[:], src_block, ident[:])
            nc.vector.tensor_scalar(out=dst, in0=ps[:], scalar1=gcol,
                                    scalar2=bcol, op0=ALU.mult, op1=ALU.add)

        ctxT = ctx.enter_context(tc.tile_pool(name="p_ctxT", bufs=1))
        ctxT_t = ctxT.tile([P, ND, SQ], F32R)               # ctx.T [(h dv), q]

        with ExitStack() as qkv_ctx:
            pqkv = qkv_ctx.enter_context(tc.tile_pool(name="p_qkv", bufs=1))
            KT = pqkv.tile([P, ND, S], F32R)                # K.T [(h dk), t]
            QT = pqkv.tile([P, ND, SQ], F32R)               # Q.T/8 [(h dk), q]
            Vaug = pqkv.tile([P, NT, H, DV + 1], F32R)      # V tokens-major + ones col

            with ExitStack() as ln_ctx:
                pln = ln_ctx.enter_context(tc.tile_pool(name="p_ln", bufs=1))
                nT = pln.tile([P, ND, S], F32R)             # LN1(xkv).T
                nqT = pln.tile([P, ND, SQ], F32R)           # LN1(xq).T

                # ---- stage A: LayerNorm1 + transpose to feature-major ----
                for tt in range(NT):
                    x_sb = io4.tile([P, D], F32, tag="xt")
                    nc.sync.dma_start(out=x_sb[:], in_=xkv_t[tt])
                    nrm = io4.tile([P, D], F32, tag="xt")
                    layernorm_tile(x_sb, nrm, P)
                    for dt in range(ND):
                        transpose_block(nT[:, dt, tt * P:(tt + 1) * P],
                                        nrm[:, dt * P:(dt + 1) * P],
                                        g1s[:, dt:dt + 1], be1s[:, dt:dt + 1])
                for tt in range(NQT):
                    x_sb = io4.tile([P, D], F32, tag="xt")
                    nc.sync.dma_start(out=x_sb[:], in_=xq_t[tt])
                    nrm = io4.tile([P, D], F32, tag="xt")
                    layernorm_tile(x_sb, nrm, P)
                    for dt in range(ND):
                        transpose_block(nqT[:, dt, tt * P:(tt + 1) * P],
                                        nrm[:, dt * P:(dt + 1) * P],
                                        g1s[:, dt:dt + 1], be1s[:, dt:dt + 1])

                # ---- stage B: Q/K/V projections ----
                for ft in range(ND):
                    wq_sb = wstream.tile([P, ND, P], F32R, tag="w128")
                    nc.sync.dma_start(out=wq_sb[:], in_=wq[ft].bitcast(F32R))
                    ps = psum_mm.tile([P, SQ], F32, tag="mm")
                    for dt in range(ND):
                        nc.tensor.matmul(ps[:], _r(wq_sb[:, dt, :]), _r(nqT[:, dt, :]),
                                         start=dt == 0, stop=dt == ND - 1)
                    # fold the 1/sqrt(DK) score scale into Q.T
                    nc.scalar.mul(out=QT[:, ft, :], in_=ps[:], mul=1.0 / np.sqrt(DK))

                    wk_sb = wstream.tile([P, ND, P], F32R, tag="w128")
                    nc.sync.dma_start(out=wk_sb[:], in_=wk[ft].bitcast(F32R))
                    for th in range(S // SQ):
                        ps = psum_mm.tile([P, SQ], F32, tag="mm")
                        for dt in range(ND):
                            nc.tensor.matmul(ps[:], _r(wk_sb[:, dt, :]),
                                             _r(nT[:, dt, th * SQ:(th + 1) * SQ]),
                                             start=dt == 0, stop=dt == ND - 1)
                        nc.scalar.copy(out=KT[:, ft, th * SQ:(th + 1) * SQ], in_=ps[:])

                # V token-major: out[t, (h dv)] in quarters of 256 features
                for fq in range(4):
                    wv_sb = wstream.tile([P, ND, 256], F32R, tag="w8k")
                    nc.sync.dma_start(out=wv_sb[:], in_=wv[fq].bitcast(F32R))
                    for tt in range(NT):
                        ps = psum_mm.tile([P, SQ], F32, tag="mm")
                        for dt in range(ND):
                            nc.tensor.matmul(ps[:, :256], _r(nT[:, dt, tt * P:(tt + 1) * P]),
                                             _r(wv_sb[:, dt, :]),
                                             start=dt == 0, stop=dt == ND - 1)
                        nc.scalar.copy(
                            out=Vaug[:, tt, fq * 4:(fq + 1) * 4, 0:DV],
                            in_=ps[:, :256].rearrange("p (h e) -> p h e", e=DV))

            nc.vector.tensor_copy(out=Vaug[:, :, :, DV:DV + 1],
                                  in_=vones[:, :, :, None])

            # ---- stage C: attention, fully transposed ----
            with ExitStack() as attn_ctx:
                pmask = attn_ctx.enter_context(tc.tile_pool(name="p_mask", bufs=1))
                mask_sb = pmask.tile([P, NT, SQ], F32)
                nc.sync.dma_start(out=mask_sb[:], in_=mask[:])
                for h in range(H):
                    hp = (h % 2) * 64
                    ff = h // 2
                    ps_c = psum_ctx.tile([DV + 1, SQ], F32, tag="ctx")
                    for kt in range(NT):
                        ps_s = psum_mm.tile([P, SQ], F32, tag="mm")
                        nc.tensor.matmul(
                            ps_s[:],
                            _r(KT[hp:hp + 64, ff, kt * P:(kt + 1) * P]),
                            _r(QT[hp:hp + 64, ff, :]),
                            start=True, stop=True)
                        pT = pt3.tile([P, SQ], F32R, tag="pt")
                        nc.scalar.activation(out=pT[:], in_=ps_s[:], func=AF.Exp)
                        nc.gpsimd.tensor_mul(pT[:], pT[:], mask_sb[:, kt, :])
                        nc.tensor.matmul(ps_c[:], _r(Vaug[:, kt, h, :]), _r(pT[:]),
                                         start=kt == 0, stop=kt == NT - 1)
                    rz = small.tile([1, SQ], F32, tag="rz")
                    nc.vector.reciprocal(out=rz[:], in_=ps_c[DV:DV + 1, :])
                    # broadcast rz across 64 partitions: ones[1,64].T @ rz[1,SQ]
                    ps_bc = psum_tr.tile([DV, SQ], F32, tag="tr")
                    nc.tensor.matmul(ps_bc[:], ones1[:], rz[:], start=True, stop=True)
                    rzb = bc2.tile([DV, SQ], F32, tag="rzb")
                    nc.scalar.copy(out=rzb[:], in_=ps_bc[:])
                    nc.vector.tensor_mul(ctxT_t[hp:hp + 64, ff, :],
                                         ps_c[0:DV, :], rzb[:])

        # ---- stages D/E/F: Wo + residual, LN2, FFN ----
        with ExitStack() as tail_ctx:
            ptail = tail_ctx.enter_context(tc.tile_pool(name="p_tail", bufs=1))
            aoT = ptail.tile([P, ND, SQ], F32)
            x1 = ptail.tile([P, NQT, D], F32)              # token-major x + attn
            n2T = ptail.tile([P, ND, SQ], F32R)
            h1T = ptail.tile([P, NM // 2, SQ], F32R)        # half of relu(h).T
            fT = ptail.tile([P, ND, SQ], F32)              # ffn output .T

            # D: attn_out.T = Wo.T @ ctx.T
            for ft in range(ND):
                wo_sb = wstream.tile([P, ND, P], F32R, tag="w128")
                nc.sync.dma_start(out=wo_sb[:], in_=wo[ft].bitcast(F32R))
                ps = psum_mm.tile([P, SQ], F32, tag="mm")
                for ct in range(ND):
                    nc.tensor.matmul(ps[:], _r(wo_sb[:, ct, :]), _r(ctxT_t[:, ct, :]),
                                     start=ct == 0, stop=ct == ND - 1)
                nc.scalar.copy(out=aoT[:, ft, :], in_=ps[:])
            # transpose back + residual -> x1 (token-major)
            for tt in range(NQT):
                xq_sb = io4.tile([P, D], F32, tag="xt")
                nc.sync.dma_start(out=xq_sb[:], in_=xq_t[tt])
                for ft in range(ND):
                    ps = psum_tr.tile([P, P], F32, tag="tr")
                    nc.tensor.transpose(ps[:], aoT[:, ft, tt * P:(tt + 1) * P], ident[:])
                    nc.vector.tensor_add(out=x1[:, tt, ft * P:(ft + 1) * P],
                                         in0=ps[:], in1=xq_sb[:, ft * P:(ft + 1) * P])

            # E: LayerNorm2 -> n2T (feature-major)
            for tt in range(NQT):
                nrm = io4.tile([P, D], F32, tag="xt")
                layernorm_tile(x1[:, tt, :], nrm, P)
                for dt in range(ND):
                    transpose_block(n2T[:, dt, tt * P:(tt + 1) * P],
                                    nrm[:, dt * P:(dt + 1) * P],
                                    g2s[:, dt:dt + 1], be2s[:, dt:dt + 1])

            # F: FFN in two DM halves to bound SBUF
            for half in range(2):
                mt0 = half * (NM // 2)
                for mi in range(NM // 2):
                    mt = mt0 + mi
                    w1_sb = wstream.tile([P, ND, P], F32R, tag="w128")
                    nc.sync.dma_start(out=w1_sb[:], in_=w1[mt].bitcast(F32R))
                    ps = psum_mm.tile([P, SQ], F32, tag="mm")
                    for dt in range(ND):
                        nc.tensor.matmul(ps[:], _r(w1_sb[:, dt, :]), _r(n2T[:, dt, :]),
                                         start=dt == 0, stop=dt == ND - 1)
                    nc.scalar.activation(out=h1T[:, mi, :], in_=ps[:], func=AF.Relu,
                                         bias=b1s[:, mt:mt + 1])
                for ft in range(ND):
                    w2_sb = wstream.tile([P, NM // 2, P], F32R, tag="w8k")
                    nc.sync.dma_start(
                        out=w2_sb[:],
                        in_=w2[ft, :, mt0:mt0 + NM // 2, :].bitcast(F32R))
                    ps = psum_mm.tile([P, SQ], F32, tag="mm")
                    for mi in range(NM // 2):
                        nc.tensor.matmul(ps[:], _r(w2_sb[:, mi, :]), _r(h1T[:, mi, :]),
                                         start=mi == 0, stop=mi == NM // 2 - 1)
                    if half == 0:
                        nc.vector.tensor_scalar(out=fT[:, ft, :], in0=ps[:],
                                                scalar1=b2s[:, ft:ft + 1], scalar2=None,
                                                op0=ALU.add)
                    else:
                        nc.vector.tensor_add(out=fT[:, ft, :], in0=fT[:, ft, :],
                                             in1=ps[:])

            # transpose back + residual -> y
            for tt in range(NQT):
                y_sb = io4.tile([P, D], F32, tag="xt")
                for ft in range(ND):
                    ps = psum_tr.tile([P, P], F32, tag="tr")
                    nc.tensor.transpose(ps[:], fT[:, ft, tt * P:(tt + 1) * P], ident[:])
                    nc.vector.tensor_add(out=y_sb[:, ft * P:(ft + 1) * P],
                                         in0=ps[:], in1=x1[:, tt, ft * P:(ft + 1) * P])
                nc.sync.dma_start(out=y_t[tt], in_=y_sb[:])

    return nc


_NC_CACHE = None


def _get_program():
    # The wait-split pass is HW-only: CoreSim's event loop cannot execute the
    # inserted carrier nops, so sim users call build_program() directly.
    global _NC_CACHE
    if _NC_CACHE is None:
        nc = build_program()
        _split_sync_waits(nc)
        _NC_CACHE = nc
    return _NC_CACHE


def make_in_maps(inputs):
    x = np.ascontiguousarray(np.asarray(inputs["x"], dtype=np.float32))
    Wq = np.asarray(inputs["Wq"], dtype=np.float32)
    Wk = np.asarray(inputs["Wk"], dtype=np.float32)
    Wv = np.asarray(inputs["Wv"], dtype=np.float32)
    def pack(w, fdim):
        # [D_in, F] -> [F//fdim, 128, D_in//128, fdim] per-tile contiguous
        din, f = w.shape
        return np.ascontiguousarray(
            w.reshape(din // P, P, f // fdim, fdim).transpose(2, 1, 0, 3))

    wq_f = pack(Wq.transpose(1, 0, 2).reshape(D, H * DK), P)
    wk_f = pack(Wk.transpose(1, 0, 2).reshape(D, H * DK), P)
    wv_f = pack(Wv.transpose(1, 0, 2).reshape(D, H * DV), 256)
    wo = pack(np.asarray(inputs["Wo"], dtype=np.float32), P)
    w1 = pack(np.asarray(inputs["w1"], dtype=np.float32), P)
    w2 = pack(np.asarray(inputs["w2"], dtype=np.float32), P)

    def col(v, n):
        return np.ascontiguousarray(
            np.asarray(v, dtype=np.float32).reshape(n, P).T)

    g1c, be1c = col(inputs["g1"], ND), col(inputs["be1"], ND)
    g2c, be2c = col(inputs["g2"], ND), col(inputs["be2"], ND)
    b1c, b2c = col(inputs["b1"], NM), col(inputs["b2"], ND)

    kk = np.arange(S).reshape(NT, P)
    masks = {}
    for q0 in (0, SQ):
        qq = q0 + np.arange(SQ)
        masks[q0] = np.ascontiguousarray(
            (qq[None, None, :] >= kk[:, :, None]).astype(np.float32)
            .transpose(1, 0, 2))

    in_maps = []
    for c in range(NCORES):
        b, q0 = c // 2, (c % 2) * SQ
        in_maps.append({
            "xkv": x[b], "xq": np.ascontiguousarray(x[b, q0:q0 + SQ]),
            "mask": masks[q0],
            "wq": wq_f, "wk": wk_f, "wv": wv_f, "wo": wo,
            "w1": w1, "w2": w2,
            "g1c": g1c, "be1c": be1c, "g2c": g2c, "be2c": be2c,
            "b1c": b1c, "b2c": b2c,
        })
    return in_maps


def kernel(**inputs):
    from concourse.bass_utils import run_bass_kernel_spmd

    nc = _get_program()
    in_maps = make_in_maps(inputs)
    res = run_bass_kernel_spmd(nc, in_maps, list(range(NCORES)))
    y = np.empty((B, S, D), dtype=np.float32)
    for c in range(NCORES):
        b, q0 = c // 2, (c % 2) * SQ
        y[b, q0:q0 + SQ] = res.results[c]["y"]
    return y
